# revision 10
# baseline (speedup 1.0000x reference)
"""Trainium2 Bass kernel for CompositionalMHA (moe_routing).

Math (see reference):
  For each bank b in {q,k,v}:  proj_b = sum_{j in top4(softmax(logits_b))}
      tw_j * (x @ U_j @ V_j)
  Then 16-head causal attention over the projections, then out @ out_w.T.

Host side: the top-k selection + softmax weights depend only on the tiny
logits vectors, so they are computed here in numpy; the selected U banks are
concatenated into [d, 4*64] and the tw-scaled V banks into [4*64, d_out].
All operands are cast to bf16 host-side (PSUM accumulation stays fp32; the
2e-2 rel-err budget dwarfs bf16 rounding).

Sharding (8 cores): core c = (batch b = c//2, head-half g = c%2).
Each core gets x[b] (transposed to [d,S]), the full U-cat per bank, the
head-half columns of V-cat per bank, and the matching 512 rows of out_w.T.
It computes a partial [S, d_model] output (its 8 heads' contribution through
the output projection); the host sums the two half-contributions per batch.

Device kernel works entirely in "transposed activation" layout [feat, S]:
  hT = Ucat^T @ xT           (contract d)
  qT/kT = Vw^T @ hT          (contract 4*64)    -> [512, S]
  v    = hT^T @ Vw           (per s-tile)       -> [S, 512] (natural layout)
  scoresT[k,q] = k_h @ q_h^T per head           -> exp -> causal mask
  outT[65, q]  = [v_h | 1]^T @ probsT           (row 64 = softmax denom)
  attnT = outT[0:64] * (1/denom broadcast across partitions)
  final[s, m] = attnT^T @ w_half                (contract feature)

Scheduling: the attention inner loop is Scalar(exp)-throughput-bound, so
score-matmul+exp staging for head-pair hp+1 is emitted before the PV chains
of hp (hp-level software pipeline), with the hT_v/v/qk projections slotted
into Scalar-busy windows as PE filler. The two heads of a pair share one
ACTIVATE over a 2-bank PSUM tile. Input DMAs are interleaved per-128-chunk
(xT/u) on the sync queue so the first hT chain starts ~1us in; second-tier
loads, softmax-denominator bounces and output stores ride the scalar
(Activation) HWDGE queue. Causally-dead columns are skipped per key-tile
(variable-N score/PV matmuls). Softmax skips max-subtraction: scores*scale
for these inputs are O(1), far from fp32 exp overflow, and softmax
normalization is scale-invariant.
"""

import numpy as np
import ml_dtypes

import concourse.bass as bass
import concourse.bacc as bacc
import concourse.mybir as mybir
import concourse.tile as tile
from concourse.bass_utils import run_bass_kernel_spmd

F32 = mybir.dt.float32
BF16 = mybir.dt.bfloat16
AF = mybir.ActivationFunctionType

P = 128
S = 1024        # sequence length
DM = 1024       # d_model
KR = 256        # top_k * r = 4 * 64
F = 512         # features per core = 8 heads * 64
NH = 8          # heads per core
HD = 64         # head dim
NG_D = DM // P  # 8
NG_R = KR // P  # 2
NG_F = F // P   # 4
NST = S // P    # 8
NSC = S // 512  # 2

TRACE = False
_cache = {}


def _emit(nc, tc, xT, us, vs, w, mask, out):
    from contextlib import ExitStack

    with ExitStack() as ctx:
        pp = ctx.enter_context(tc.tile_pool(name="persist", bufs=1))

        xT_sb = pp.tile([P, NG_D, S], BF16)
        u_sb = {b: pp.tile([P, NG_D, KR], BF16, name=f"u{b}_sb") for b in "qkv"}
        vw_sb = {b: pp.tile([P, NG_R, F], BF16, name=f"vw{b}_sb") for b in "qkv"}
        mask_sb = pp.tile([P, P], BF16)
        w_sb = pp.tile([P, NG_F, DM], BF16)

        # sync queue: exactly what the first hT bank needs, interleaved
        # per-chunk so the first accumulation chain starts ~1us in.
        for g in range(NG_D):
            nc.sync.dma_start(out=xT_sb[:, g, :], in_=xT[g * P:(g + 1) * P, :])
            nc.sync.dma_start(out=u_sb["q"][:, g, :], in_=us["q"][g * P:(g + 1) * P, :])
        for b in "qk":
            for g in range(NG_R):
                nc.sync.dma_start(out=vw_sb[b][:, g, :], in_=vs[b][g * P:(g + 1) * P, :])
        # scalar queue: second tier (mask first: needed by hp0 staging).
        nc.scalar.dma_start(out=mask_sb, in_=mask)
        for g in range(NG_D):
            nc.scalar.dma_start(out=u_sb["k"][:, g, :], in_=us["k"][g * P:(g + 1) * P, :])
        for g in range(NG_F):
            nc.scalar.dma_start(out=w_sb[:, g, :], in_=w[g * P:(g + 1) * P, :])
        # gpsimd (SWDGE) queue: the v-bank, needed only mid-schedule.
        for g in range(NG_D):
            nc.gpsimd.dma_start(out=u_sb["v"][:, g, :], in_=us["v"][g * P:(g + 1) * P, :])
        for g in range(NG_R):
            nc.gpsimd.dma_start(out=vw_sb["v"][:, g, :], in_=vs["v"][g * P:(g + 1) * P, :])

        qT_sb = pp.tile([P, NG_F, S], BF16)
        kT_sb = pp.tile([P, NG_F, S], BF16)
        vS_sb = pp.tile([P, NST, NH, HD + 1], BF16)
        nc.vector.memset(vS_sb[:, :, :, HD:HD + 1], 1.0)
        attnT_sb = pp.tile([P, NG_F, S], BF16)
        # softmax denominators: row di lives at partition 32*(di%4),
        # free slot di//4 (output base-partition must be 32-aligned)
        den_sb = pp.tile([P, 4, 512], F32)
        nc.vector.memset(den_sb, 1.0)
        den_dram = nc.dram_tensor("den_scratch", [16, 512], F32,
                                  kind="Internal").ap()

        hT_sb = {}
        hpool = ctx.enter_context(tc.tile_pool(name="hpool", bufs=3))
        spp = ctx.enter_context(tc.tile_pool(name="spp", bufs=26))
        spr = ctx.enter_context(tc.tile_pool(name="spr", bufs=4))
        spo = ctx.enter_context(tc.tile_pool(name="spo", bufs=3))

        with (
            tc.tile_pool(name="pps", bufs=3, space="PSUM") as pps,
            tc.tile_pool(name="ppo", bufs=2, space="PSUM") as ppo,
        ):
            def emit_hT(b):
                # g-outer accumulation across 4 parallel PSUM chains so the
                # first matmul fires as soon as chunk g=0 lands in SBUF.
                hT_sb[b] = hpool.tile([P, NG_R, S], BF16, name=f"hT_{b}", tag="hT")
                h_ps = [pps.tile([P, NSC, 512], F32, name=f"h_ps{mi}", tag="s_ps")
                        for mi in range(NG_R)]
                for g in range(NG_D):
                    for mi in range(NG_R):
                        for sc in range(NSC):
                            nc.tensor.matmul(
                                h_ps[mi][:, sc, :],
                                lhsT=u_sb[b][:, g, mi * P:(mi + 1) * P],
                                rhs=xT_sb[:, g, sc * 512:(sc + 1) * 512],
                                start=(g == 0), stop=(g == NG_D - 1))
                for mi in range(NG_R):
                    for sc in range(NSC):
                        nc.vector.tensor_copy(
                            hT_sb[b][:, mi, sc * 512:(sc + 1) * 512],
                            h_ps[mi][:, sc, :])

            def emit_v():
                for sp in range(NST // 2):
                    v_ps = pps.tile([P, 2, F], F32, name="v_ps", tag="s_ps")
                    for half in range(2):
                        st = 2 * sp + half
                        for mi in range(NG_R):
                            nc.tensor.matmul(
                                v_ps[:, half, :],
                                lhsT=hT_sb["v"][:, mi, st * P:(st + 1) * P],
                                rhs=vw_sb["v"][:, mi, :],
                                start=(mi == 0), stop=(mi == NG_R - 1))
                    for half in range(2):
                        st = 2 * sp + half
                        nc.vector.tensor_copy(
                            vS_sb[:, st, :, 0:HD],
                            v_ps[:, half, :].rearrange("p (h e) -> p h e", h=NH))

            def emit_qk(fc):
                # qT/kT feature chunk fc (heads 2fc, 2fc+1) from hT
                for b in "qk":
                    dst = qT_sb if b == "q" else kT_sb
                    b_ps = pps.tile([P, 2, 512], F32, name="b_ps", tag="s_ps")
                    for sc in range(NSC):
                        for mi in range(NG_R):
                            nc.tensor.matmul(
                                b_ps[:, sc, :],
                                lhsT=vw_sb[b][:, mi, fc * P:(fc + 1) * P],
                                rhs=hT_sb[b][:, mi, sc * 512:(sc + 1) * 512],
                                start=(mi == 0), stop=(mi == NG_R - 1))
                    for sc in range(NSC):
                        nc.vector.tensor_copy(
                            dst[:, fc, sc * 512:(sc + 1) * 512], b_ps[:, sc, :])

            TILES = [(qc, kt) for qc in range(NSC) for kt in range(4 * (qc + 1))]

            def emit_stage_tile(hp, qc, kt, pT):
                rel = P * kt - 512 * qc
                q0 = max(rel, 0)
                s_ps = pps.tile([P, 2, 512], F32, name="s_ps", tag="s_ps")
                for sub in range(2):
                    po = HD * sub
                    nc.tensor.matmul(
                        s_ps[:, sub, q0:512],
                        lhsT=kT_sb[po:po + HD, hp, kt * P:(kt + 1) * P],
                        rhs=qT_sb[po:po + HD, hp, qc * 512 + q0:(qc + 1) * 512],
                        start=True, stop=True)
                t = spp.tile([P, 2, 512], BF16, name="pT", tag="pT")
                pT[(qc, kt)] = t
                nc.scalar.activation(
                    out=t[:, :, q0:512], in_=s_ps[:, :, q0:512],
                    func=AF.Exp, scale=0.125)
                if rel >= 0:
                    # causal-crossing tile: cols [q0, q0+128) need the
                    # triangular mask; cols < q0 are never read.
                    for sub in range(2):
                        nc.gpsimd.tensor_mul(
                            t[:, sub, q0:q0 + P], t[:, sub, q0:q0 + P],
                            mask_sb)

            def emit_pv_tile(hp, qc, kt, pT, o_ps):
                n_kt = 4 * (qc + 1)
                q0 = max(P * kt - 512 * qc, 0)
                for sub in range(2):
                    h = 2 * hp + sub
                    nc.tensor.matmul(
                        o_ps[sub][:, q0:512],
                        lhsT=vS_sb[:, kt, h, :],
                        rhs=pT[(qc, kt)][:, sub, q0:512],
                        start=(kt == 0), stop=(kt == n_kt - 1))

            def finish_qc(hp, qc, o_ps):
                for sub in range(2):
                    po = HD * sub
                    di = (hp * 2 + qc) * 2 + sub
                    nc.vector.tensor_copy(
                        attnT_sb[po:po + HD, hp, qc * 512:(qc + 1) * 512],
                        o_ps[sub][0:HD, :])
                    nc.vector.tensor_copy(
                        den_sb[32 * (di % 4):32 * (di % 4) + 1, di // 4, :],
                        o_ps[sub][HD:HD + 1, :])
                norm_qc(hp, qc)

            def norm_qc(hp, qc):
                # den rows di = 4*hp + 2*qc + sub live in free slot hp
                rcp = spr.tile([P, 512], F32, name="rcp", tag="rcp", bufs=2)
                nc.vector.reciprocal_approx_fast(out=rcp, in_=den_sb[:, hp, :])
                for sub in range(2):
                    di = 4 * hp + 2 * qc + sub
                    po = HD * sub
                    bc_sb = spr.tile([P, 512], F32, name="bc_sb", tag="bc_sb")
                    # HW partition_broadcast ignores AP offsets, so bounce the
                    # reciprocal row through DRAM and broadcast-load it
                    # (stride-0 partition APs are legal for DRAM sources).
                    nc.scalar.dma_start(
                        out=den_dram[di:di + 1, :],
                        in_=rcp[32 * (di % 4):32 * (di % 4) + 1, :])
                    nc.scalar.dma_start(
                        out=bc_sb,
                        in_=bass.AP(
                            tensor=den_dram.tensor,
                            offset=den_dram[di:di + 1, :].offset,
                            ap=[[0, P], [1, 512]]))
                    sl = attnT_sb[po:po + HD, hp, qc * 512:(qc + 1) * 512]
                    nc.vector.tensor_mul(sl, sl, bc_sb[po:po + HD, :])

            def stage(hp):
                pT = {}
                for (qc, kt) in TILES:
                    emit_stage_tile(hp, qc, kt, pT)
                return pT

            def stage_and_pv(hp_next, hp, pT):
                # interleave staging of hp_next with PV chains of hp at tile
                # granularity: the PV matmuls (probs long since ready) fill
                # the PE stalls where staging waits on the exp pipeline.
                pT_next = {}
                o_ps = None
                cur_qc = -1
                for i, (qc, kt) in enumerate(TILES):
                    if hp_next is not None:
                        emit_stage_tile(hp_next, qc, kt, pT_next)
                    if qc != cur_qc:
                        if o_ps is not None:
                            finish_qc(hp, cur_qc, o_ps)
                        o_ps = [
                            ppo.tile([HD + 1, 512], F32, name=f"o_ps{s_}",
                                     tag="o_ps")
                            for s_ in range(2)
                        ]
                        cur_qc = qc
                    emit_pv_tile(hp, qc, kt, pT, o_ps)
                finish_qc(hp, cur_qc, o_ps)
                return pT_next

            # ---- schedule ----
            emit_hT("q")
            emit_hT("k")
            emit_qk(0)
            pT = stage(0)
            emit_hT("v")
            emit_v()
            emit_qk(1)
            pT = stage_and_pv(1, 0, pT)
            emit_qk(2)
            pT = stage_and_pv(2, 1, pT)
            emit_qk(3)
            pT = stage_and_pv(3, 2, pT)
            stage_and_pv(None, 3, pT)

        # ---- Phase D: output projection ----
        with tc.tile_pool(name="ppf", bufs=8, space="PSUM") as ppf:
            for st in range(NST):
                for mc in range(NSC):
                    f_ps = ppf.tile([P, 512], F32, name="f_ps", tag="f_ps")
                    for fcc in range(NG_F):
                        nc.tensor.matmul(
                            f_ps,
                            lhsT=attnT_sb[:, fcc, st * P:(st + 1) * P],
                            rhs=w_sb[:, fcc, mc * 512:(mc + 1) * 512],
                            start=(fcc == 0), stop=(fcc == NG_F - 1))
                    o_sb = spo.tile([P, 512], F32, name="o_sb", tag="o_sb")
                    if (st * NSC + mc) % 2 == 0:
                        nc.scalar.copy(out=o_sb, in_=f_ps)
                        eng = nc.scalar
                    else:
                        nc.vector.tensor_copy(o_sb, f_ps)
                        eng = nc.sync
                    eng.dma_start(
                        out=out[st * P:(st + 1) * P, mc * 512:(mc + 1) * 512],
                        in_=o_sb)


def _build():
    nc = bacc.Bacc("TRN2", target_bir_lowering=False, debug=False, num_devices=8)
    xT = nc.dram_tensor("xT", [DM, S], BF16, kind="ExternalInput").ap()
    us = {b: nc.dram_tensor(f"u{b}", [DM, KR], BF16, kind="ExternalInput").ap()
          for b in "qkv"}
    vs = {b: nc.dram_tensor(f"v{b}", [KR, F], BF16, kind="ExternalInput").ap()
          for b in "qkv"}
    w = nc.dram_tensor("w", [F, DM], BF16, kind="ExternalInput").ap()
    mask = nc.dram_tensor("mask", [P, P], BF16, kind="ExternalInput").ap()
    out = nc.dram_tensor("out", [S, DM], F32, kind="ExternalOutput").ap()
    with tile.TileContext(nc) as tc:
        _emit(nc, tc, xT, us, vs, w, mask, out)
    nc.compile()
    return nc


def _tri_mask():
    # tri[rk, c] = 1.0 iff c >= rk  (keep where key index <= query index
    # within a diagonal 128x128 block)
    rk = np.arange(P)[:, None]
    c = np.arange(P)[None, :]
    return (c >= rk).astype(ml_dtypes.bfloat16)


def _select_bank(U, V, logits, top_k):
    lg = np.asarray(logits, np.float32)
    e = np.exp(lg - lg.max())
    wsoft = (e / e.sum()).astype(np.float32)
    ti = np.argsort(-wsoft, kind="stable")[:top_k]
    tw = wsoft[ti]
    tw = tw / tw.sum()
    Ucat = np.concatenate([U[i] for i in ti], axis=1)          # [d, k*r]
    Vcat = np.concatenate([tw[k] * V[ti[k]] for k in range(top_k)], axis=0)
    return (np.ascontiguousarray(Ucat).astype(ml_dtypes.bfloat16),
            np.ascontiguousarray(Vcat).astype(ml_dtypes.bfloat16))


def kernel(**inputs):
    x = np.asarray(inputs["x"], np.float32)          # [4, S, d]
    out_w = np.asarray(inputs["out_w"], np.float32)  # [d, d]
    top_k = int(np.asarray(inputs["top_k"]))
    assert top_k * 64 == KR, f"kernel compiled for top_k=4, got {top_k}"
    B = x.shape[0]

    cats = {}
    for b in "qkv":
        cats[b] = _select_bank(
            np.asarray(inputs[f"{b}_U"], np.float32),
            np.asarray(inputs[f"{b}_V"], np.float32),
            inputs[f"{b}_logits"], top_k)

    if "nc" not in _cache:
        _cache["nc"] = _build()
    nc = _cache["nc"]

    mask = _tri_mask()
    wT = np.ascontiguousarray(out_w.T).astype(ml_dtypes.bfloat16)
    in_maps = []
    for c in range(8):
        b, g = c // 2, c % 2
        m = {"xT": np.ascontiguousarray(x[b].T).astype(ml_dtypes.bfloat16),
             "mask": mask,
             "w": np.ascontiguousarray(wT[g * F:(g + 1) * F, :])}
        for bank in "qkv":
            Ucat, Vcat = cats[bank]
            m[f"u{bank}"] = Ucat
            m[f"v{bank}"] = np.ascontiguousarray(Vcat[:, g * F:(g + 1) * F])
        in_maps.append(m)

    res = run_bass_kernel_spmd(nc, in_maps, core_ids=list(range(8)), trace=TRACE)
    if TRACE:
        _cache["last_results"] = res
    parts = [r["out"] for r in res.results]
    full = np.stack([parts[2 * b] + parts[2 * b + 1] for b in range(B)])
    return full.astype(np.float32)
